# revision 4
# baseline (speedup 1.0000x reference)
"""Llama4-style MoE experts (grouped SwiGLU MLP) on Trainium2, 8 NeuronCores.

Expert-parallel: core i runs expert i's full MLP on its 1024-token slice:
    out = (up * silu(gate)) @ W2,  [gate|up] = h @ W1
Per-core shapes: h [1024, 2048], W1 [2048, 8192], W2 [4096, 2048].

v2 schedule, engineered so the TensorEngine does nothing but matmuls:
- h is transposed by the DMA engine (XBAR transpose of the bf16 rows)
  straight into the [h, t] layout mm1 needs -- no PE transposes, no
  PSUM traffic for them.
- W1 is cast f32->bf16 on the VectorEngine in [128,512] stripes into a
  ring that stays a full f-block ahead of the PE, so mm1 never waits on
  casts in steady state.
- SwiGLU uses the ScalarEngine's fused Silu (one activation + one
  vector multiply per tile).
- mm2 runs f-outer / tt-inner with all 8 PSUM banks accumulating, W2
  stripes DMA+cast software-pipelined one hb-block ahead; output drains
  via ScalarEngine copies so the VectorEngine only feeds W2 casts.
- Two hardware DMA queues: SP carries W1/W2/out, Activation carries
  h rows + transposes.
"""

from contextlib import ExitStack

import numpy as np

import concourse.bass as bass
import concourse.mybir as mybir
import concourse.tile as tile
from concourse import bacc
from concourse.bass_utils import run_bass_kernel_spmd

N_CORES = 8
P = 128
TB = 512  # moving-operand free-dim block (one PSUM bank of f32)

F32 = mybir.dt.float32
BF16 = mybir.dt.bfloat16
ACT_SILU = mybir.ActivationFunctionType.Silu
ACT_COPY = mybir.ActivationFunctionType.Copy

# Per-core problem dims (full problem: 8 experts x 1024 tokens, H=2048, F=4096)
T = 1024
H = 2048
F = 4096


def build_kernel_body(tc, T=T, H=H, F=F):
    nc = tc.nc
    h_d = nc.dram_tensor("hidden_states", [T, H], F32, kind="ExternalInput").ap()
    w1_d = nc.dram_tensor("gate_up_proj", [H, 2 * F], F32, kind="ExternalInput").ap()
    w2_d = nc.dram_tensor("down_proj", [F, H], F32, kind="ExternalInput").ap()
    out_d = nc.dram_tensor("out", [T, H], F32, kind="ExternalOutput").ap()

    n_hh = H // P          # 16 contraction tiles of mm1
    n_fb = F // TB         # 8 512-wide f blocks of W1 (per gate/up half)
    n_if = TB // P         # 4 f-tiles per block
    n_tb = T // TB         # 2 token blocks
    n_tt = T // P          # 8 token tiles (mm2 psum partitions)
    n_ft = F // P          # 32 f-tiles (mm2 contraction)
    n_hb = H // TB         # 4 output column blocks
    HC = 1024              # h-row DMA/cast/transpose chunk width
    n_hc = H // HC         # 2 chunks per token row

    with ExitStack() as ctx:
        htp = ctx.enter_context(tc.tile_pool(name="htp", bufs=1))
        actp = ctx.enter_context(tc.tile_pool(name="actp", bufs=1))
        hrowp = ctx.enter_context(tc.tile_pool(name="hrowp", bufs=2))
        wfp = ctx.enter_context(tc.tile_pool(name="wfp", bufs=5))
        w1bp = ctx.enter_context(tc.tile_pool(name="w1bp", bufs=33))
        w2fp = ctx.enter_context(tc.tile_pool(name="w2fp", bufs=5))
        b2p = ctx.enter_context(tc.tile_pool(name="b2p", bufs=33))
        sgp = ctx.enter_context(tc.tile_pool(name="sgp", bufs=3))
        outp = ctx.enter_context(tc.tile_pool(name="outp", bufs=4))
        ps = ctx.enter_context(tc.tile_pool(name="ps", bufs=8, space="PSUM"))

        # [h, t] bf16 layout of h: ht[:, hh*T + t] = h[t, hh*P + p]
        ht = htp.tile([P, n_hh * T], BF16, tag="ht", name="ht")
        ht3 = ht[:, :].rearrange("p (hh t) -> p hh t", hh=n_hh)
        # act[f, t] bf16: act[:, fi*T + t] = act value at (fi*P + p, t)
        act = actp.tile([P, n_ft * T], BF16, tag="act", name="act")

        def ht_rhs(hh, tb):
            return ht[:, hh * T + tb * TB : hh * T + (tb + 1) * TB]

        # ---- Phase A: h rows -> bf16 -> DMA-XBAR transpose into ht ----
        # Row DMAs + transposes ride the Activation queue; W1 owns SP.
        hrows = {}
        for ti in range(n_tt):
            for c in range(n_hc):
                hr = hrowp.tile([P, HC], F32, tag="hrow", name=f"hr{ti}_{c}")
                nc.scalar.dma_start(
                    hr[:], h_d[ti * P : (ti + 1) * P, c * HC : (c + 1) * HC]
                )
                hrows[(ti, c)] = hr

        def cast_h(ti, c):
            hb = hrowp.tile([P, HC], BF16, tag="hb16", name=f"hb{ti}_{c}")
            nc.vector.tensor_copy(out=hb[:], in_=hrows[(ti, c)][:])
            hrows[(ti, c)] = hb

        def transpose_h(ti, c):
            nhh = HC // P
            dst = ht3[:, c * nhh : (c + 1) * nhh, ti * P : (ti + 1) * P]
            nc.scalar.dma_start_transpose(dst, hrows[(ti, c)][:])

        # ---- W1 streaming: DMA f32 stripes (SP queue) + DVE casts ----
        def load_w1_block(fb):
            """Issue DMAs for one 512-wide f-block (gate + up halves)."""
            st = {}
            for hh in range(n_hh):
                for half, c0 in (("g", fb * TB), ("u", F + fb * TB)):
                    s = wfp.tile([P, TB], F32, tag="wf", name=f"w1f_{fb}_{hh}_{half}")
                    nc.sync.dma_start(
                        s[:], w1_d[hh * P : (hh + 1) * P, c0 : c0 + TB]
                    )
                    st[(hh, half)] = s
            return st

        def cast_w1_block(fb, st):
            wb = {}
            for hh in range(n_hh):
                for half in ("g", "u"):
                    b = w1bp.tile([P, TB], BF16, tag="w1b", name=f"w1b_{fb}_{hh}_{half}")
                    nc.vector.tensor_copy(out=b[:], in_=st[(hh, half)][:])
                    wb[(hh, half)] = b
            return wb

        def swiglu(fi, tb, pg, pu):
            sg = sgp.tile([P, TB], BF16, tag="sg", name=f"sg{fi}_{tb}")
            nc.scalar.activation(sg[:], pg[:], ACT_SILU)
            nc.vector.tensor_mul(
                out=act[:, fi * T + tb * TB : fi * T + (tb + 1) * TB],
                in0=pu[:],
                in1=sg[:],
            )

        def mm1_i_major(fb, wb):
            """One f-block, i-major: per i, 4 psum banks, sweep all hh."""
            for i in range(n_if):
                fi = fb * n_if + i
                pg = [ps.tile([P, TB], F32, tag="ps", name=f"pg{fi}_{tb}")
                      for tb in range(n_tb)]
                pu = [ps.tile([P, TB], F32, tag="ps", name=f"pu{fi}_{tb}")
                      for tb in range(n_tb)]
                for hh in range(n_hh):
                    first, last = hh == 0, hh == n_hh - 1
                    for half, pp in (("g", pg), ("u", pu)):
                        lhsT = wb[(hh, half)][:, i * P : (i + 1) * P]
                        for tb in range(n_tb):
                            nc.tensor.matmul(
                                pp[tb][:], lhsT=lhsT, rhs=ht_rhs(hh, tb),
                                start=first, stop=last,
                            )
                for tb in range(n_tb):
                    swiglu(fi, tb, pg[tb], pu[tb])

        def mm1_fb0(wb):
            """fb0, tb-split + hh-outer so matmuls pace off the arriving
            h transposes and W1 cast stream (startup pipelining)."""
            for tb in range(n_tb):
                pg = [ps.tile([P, TB], F32, tag="ps", name=f"z_pg{i}_{tb}")
                      for i in range(n_if)]
                pu = [ps.tile([P, TB], F32, tag="ps", name=f"z_pu{i}_{tb}")
                      for i in range(n_if)]
                for hh in range(n_hh):
                    first, last = hh == 0, hh == n_hh - 1
                    for i in range(n_if):
                        for half, pp in (("g", pg), ("u", pu)):
                            lhsT = wb[(hh, half)][:, i * P : (i + 1) * P]
                            nc.tensor.matmul(
                                pp[i][:], lhsT=lhsT, rhs=ht_rhs(hh, tb),
                                start=first, stop=last,
                            )
                for i in range(n_if):
                    swiglu(i, tb, pg[i], pu[i])

        # ---- Phase B emission ----
        # DMAs for fb0 first on SP, then h casts (DVE) interleaved with
        # fb0 casts so neither blocks the other's consumer.
        st0 = load_w1_block(0)
        for ti in range(4):
            for c in range(n_hc):
                cast_h(ti, c)
        for ti in range(4):
            for c in range(n_hc):
                transpose_h(ti, c)
        wb0 = cast_w1_block(0, st0)
        for ti in range(4, n_tt):
            for c in range(n_hc):
                cast_h(ti, c)
        for ti in range(4, n_tt):
            for c in range(n_hc):
                transpose_h(ti, c)

        st1 = load_w1_block(1)
        mm1_fb0(wb0)
        wb = cast_w1_block(1, st1)
        for fb in range(1, n_fb):
            if fb + 1 < n_fb:
                st_next = load_w1_block(fb + 1)
            mm1_i_major(fb, wb)
            if fb + 1 < n_fb:
                wb = cast_w1_block(fb + 1, st_next)

        # ---- W2 hb0 prefetch (lands during fb7 compute) ----
        def w2_stripe(hb, f):
            s = w2fp.tile([P, TB], F32, tag="w2f", name=f"w2f_{hb}_{f}")
            nc.sync.dma_start(
                s[:], w2_d[f * P : (f + 1) * P, hb * TB : (hb + 1) * TB]
            )
            b = b2p.tile([P, TB], BF16, tag="b2", name=f"b2_{hb}_{f}")
            nc.vector.tensor_copy(out=b[:], in_=s[:])
            return b

        b2 = {}
        for f in range(n_ft):
            b2[(0, f)] = w2_stripe(0, f)

        # ---- Phase C: out = act @ W2, f-outer / tt-inner, 8 psum banks ----
        for hb in range(n_hb):
            po = [ps.tile([P, TB], F32, tag="ps", name=f"po{hb}_{tt}")
                  for tt in range(n_tt)]
            for f in range(n_ft):
                if hb + 1 < n_hb:
                    b2[(hb + 1, f)] = w2_stripe(hb + 1, f)
                first, last = f == 0, f == n_ft - 1
                rhs = b2[(hb, f)][:]
                for tt in range(n_tt):
                    lhsT = act[:, f * T + tt * P : f * T + (tt + 1) * P]
                    nc.tensor.matmul(
                        po[tt][:], lhsT=lhsT, rhs=rhs, start=first, stop=last
                    )
            for tt in range(n_tt):
                ob = outp.tile([P, TB], F32, tag="outp", name=f"ob{hb}_{tt}")
                nc.scalar.activation(ob[:], po[tt][:], ACT_COPY)
                nc.sync.dma_start(
                    out_d[tt * P : (tt + 1) * P, hb * TB : (hb + 1) * TB], ob[:]
                )


def build_nc(T=T, H=H, F=F):
    nc = bacc.Bacc(
        "TRN2", target_bir_lowering=False, debug=False, enable_asserts=False
    )
    with tile.TileContext(nc) as tc:
        build_kernel_body(tc, T=T, H=H, F=F)
    nc.compile()
    return nc


_NC_CACHE = None


def run(hidden_states, gate_up_proj, down_proj, trace=False, **kw):
    """Run on the 8 NeuronCores; returns (output, BassKernelResults)."""
    global _NC_CACHE
    if _NC_CACHE is None:
        _NC_CACHE = build_nc()
    nc = _NC_CACHE

    hs = np.ascontiguousarray(np.asarray(hidden_states), dtype=np.float32)
    gup = np.ascontiguousarray(np.asarray(gate_up_proj), dtype=np.float32)
    dp = np.ascontiguousarray(np.asarray(down_proj), dtype=np.float32)
    assert hs.shape == (N_CORES * T, H), hs.shape
    assert gup.shape == (N_CORES, H, 2 * F), gup.shape
    assert dp.shape == (N_CORES, F, H), dp.shape

    in_maps = [
        {
            "hidden_states": np.ascontiguousarray(hs[i * T : (i + 1) * T]),
            "gate_up_proj": np.ascontiguousarray(gup[i]),
            "down_proj": np.ascontiguousarray(dp[i]),
        }
        for i in range(N_CORES)
    ]
    res = run_bass_kernel_spmd(
        nc, in_maps, core_ids=list(range(N_CORES)), trace=trace, **kw
    )
    out = np.concatenate(
        [res.results[i]["out"] for i in range(N_CORES)], axis=0
    ).astype(np.float32)
    return out, res


def kernel(hidden_states, gate_up_proj, down_proj):
    out, _ = run(hidden_states, gate_up_proj, down_proj, trace=False)
    return out


# revision 7
# speedup vs baseline: 1.0527x; 1.0527x over previous
"""Llama4-style MoE experts (grouped SwiGLU MLP) on Trainium2, 8 NeuronCores.

Expert-parallel: core i runs expert i's full MLP on its 1024-token slice:
    out = (up * silu(gate)) @ W2,  [gate|up] = h @ W1
Per-core shapes: h [1024, 2048], W1 [2048, 8192], W2 [4096, 2048].

v3 schedule, engineered so the TensorEngine does nothing but matmuls:
- h is transposed by the DMA engine (XBAR transpose of the bf16 rows)
  straight into the [h, t] layout mm1 needs -- no PE transposes.
- W1 streams in 256-wide f-blocks; each (hh, block) is ONE fused
  gate+up DMA and one [128,512] VectorEngine cast.  The bf16 ring holds
  exactly two blocks, and block k+1's casts are emitted between block
  k's i-groups, so in steady state the PE never waits on W1.
- Startup: h rows are DMA-prioritized (Activation queue) over W1 (SP
  queue); the first f-block runs hh-outer so matmuls pace off the
  arriving stripe casts.
- SwiGLU uses the ScalarEngine's fused Silu (one activation + one
  vector multiply per tile).
- mm2 runs f-outer / tt-inner with all 8 PSUM banks accumulating, W2
  stripes DMA+cast software-pipelined ahead; output drains via
  ScalarEngine copies; the last hb runs tt-outer so the drain staggers.
"""

from contextlib import ExitStack

import numpy as np

import concourse.bass as bass
import concourse.mybir as mybir
import concourse.tile as tile
from concourse import bacc
from concourse.bass_utils import run_bass_kernel_spmd

N_CORES = 8
P = 128
TB = 512   # token/psum free-dim block (one PSUM bank of f32)
WF = 256   # W1 f-block width

F32 = mybir.dt.float32
BF16 = mybir.dt.bfloat16
ACT_SILU = mybir.ActivationFunctionType.Silu
ACT_COPY = mybir.ActivationFunctionType.Copy

# Per-core problem dims (full problem: 8 experts x 1024 tokens, H=2048, F=4096)
T = 1024
H = 2048
F = 4096


def build_kernel_body(tc, T=T, H=H, F=F):
    nc = tc.nc
    h_d = nc.dram_tensor("hidden_states", [T, H], F32, kind="ExternalInput").ap()
    w1_d = nc.dram_tensor("gate_up_proj", [H, 2 * F], F32, kind="ExternalInput").ap()
    w2_d = nc.dram_tensor("down_proj", [F, H], F32, kind="ExternalInput").ap()
    out_d = nc.dram_tensor("out", [T, H], F32, kind="ExternalOutput").ap()

    n_hh = H // P          # 16 contraction tiles of mm1
    n_fb = F // WF         # 16 256-wide f blocks of W1 (per gate/up half)
    n_if = WF // P         # 2 f-tiles per block
    n_tb = T // TB         # 2 token blocks
    n_tt = T // P          # 8 token tiles (mm2 psum partitions)
    n_ft = F // P          # 32 f-tiles (mm2 contraction)
    n_hb = H // TB         # 4 output column blocks
    HC = 1024              # h-row DMA/cast/transpose chunk width
    n_hc = H // HC         # 2 chunks per token row

    # gate|up halves of W1 as [H, 2, F] so one DMA fetches both halves
    w1_3d = w1_d.rearrange("h (half f) -> h half f", half=2)

    with ExitStack() as ctx:
        htp = ctx.enter_context(tc.tile_pool(name="htp", bufs=1))
        actp = ctx.enter_context(tc.tile_pool(name="actp", bufs=1))
        hrowp = ctx.enter_context(tc.tile_pool(name="hrowp", bufs=2))
        wfp = ctx.enter_context(tc.tile_pool(name="wfp", bufs=6))
        w1bp = ctx.enter_context(tc.tile_pool(name="w1bp", bufs=32))
        w2fp = ctx.enter_context(tc.tile_pool(name="w2fp", bufs=6))
        b2p = ctx.enter_context(tc.tile_pool(name="b2p", bufs=18))
        sgp = ctx.enter_context(tc.tile_pool(name="sgp", bufs=4))
        outp = ctx.enter_context(tc.tile_pool(name="outp", bufs=6))
        ps = ctx.enter_context(tc.tile_pool(name="ps", bufs=8, space="PSUM"))

        # [h, t] bf16 layout of h: ht[:, hh*T + t] = h[t, hh*P + p]
        ht = htp.tile([P, n_hh * T], BF16, tag="ht", name="ht")
        ht3 = ht[:, :].rearrange("p (hh t) -> p hh t", hh=n_hh)
        # act[f, t] bf16: act[:, fi*T + t] = act value at (fi*P + p, t)
        act = actp.tile([P, n_ft * T], BF16, tag="act", name="act")

        def ht_rhs(hh, tb):
            return ht[:, hh * T + tb * TB : hh * T + (tb + 1) * TB]

        # ---- h path: rows -> bf16 -> DMA-XBAR transpose into ht ----
        hrows = {}

        def load_h(ti, c):
            hr = hrowp.tile([P, HC], F32, tag="hrow", name=f"hr{ti}_{c}")
            nc.scalar.dma_start(
                hr[:], h_d[ti * P : (ti + 1) * P, c * HC : (c + 1) * HC]
            )
            hrows[(ti, c)] = hr

        def cast_h(ti, c):
            hb = hrowp.tile([P, HC], BF16, tag="hb16", name=f"hb{ti}_{c}")
            nc.vector.tensor_copy(out=hb[:], in_=hrows[(ti, c)][:])
            hrows[(ti, c)] = hb

        def transpose_h(ti, c):
            nhh = HC // P
            dst = ht3[:, c * nhh : (c + 1) * nhh, ti * P : (ti + 1) * P]
            nc.scalar.dma_start_transpose(dst, hrows[(ti, c)][:])

        # ---- W1 streaming: fused gate+up stripe DMA (SP) + DVE cast ----
        def load_w1_block(fb):
            """One 256-wide f-block: per hh, one [128, 2, 256] DMA."""
            st = []
            for hh in range(n_hh):
                s = wfp.tile([P, 2 * WF], F32, tag="wf", name=f"w1f_{fb}_{hh}")
                s3 = s[:, :].rearrange("p (half f) -> p half f", half=2)
                nc.sync.dma_start(
                    s3, w1_3d[hh * P : (hh + 1) * P, :, fb * WF : (fb + 1) * WF]
                )
                st.append(s)
            return st

        def cast_w1(fb, st, hh):
            b = w1bp.tile([P, 2 * WF], BF16, tag="w1b", name=f"w1b_{fb}_{hh}")
            nc.vector.tensor_copy(out=b[:], in_=st[hh][:])
            return b

        def w1_lhsT(wb, hh, half, i):
            off = half * WF + i * P
            return wb[hh][:, off : off + P]

        def swiglu(fi, tb, pg, pu):
            sg = sgp.tile([P, TB], BF16, tag="sg", name=f"sg{fi}_{tb}")
            nc.scalar.activation(sg[:], pg[:], ACT_SILU)
            nc.vector.tensor_mul(
                out=act[:, fi * T + tb * TB : fi * T + (tb + 1) * TB],
                in0=pu[:],
                in1=sg[:],
            )

        def mm1_i_major(fb, wb, cast_mid=None):
            """One f-block, i-major: per i, 4 psum banks, sweep all hh.
            cast_mid(i) lets the caller emit the next block's casts
            between i-groups so they run during this block's sweeps."""
            for i in range(n_if):
                fi = fb * n_if + i
                pg = [ps.tile([P, TB], F32, tag="ps", name=f"pg{fi}_{tb}")
                      for tb in range(n_tb)]
                pu = [ps.tile([P, TB], F32, tag="ps", name=f"pu{fi}_{tb}")
                      for tb in range(n_tb)]
                for hh in range(n_hh):
                    first, last = hh == 0, hh == n_hh - 1
                    for half, pp in ((0, pg), (1, pu)):
                        lhsT = w1_lhsT(wb, hh, half, i)
                        for tb in range(n_tb):
                            nc.tensor.matmul(
                                pp[tb][:], lhsT=lhsT, rhs=ht_rhs(hh, tb),
                                start=first, stop=last,
                            )
                for tb in range(n_tb):
                    swiglu(fi, tb, pg[tb], pu[tb])
                if cast_mid is not None:
                    cast_mid(i)

        def mm1_fb0(wb, cast_mid):
            """fb0, tb-split + hh-outer so matmuls pace off the arriving
            h transposes and W1 cast stream (startup pipelining)."""
            for tb in range(n_tb):
                pg = [ps.tile([P, TB], F32, tag="ps", name=f"z_pg{i}_{tb}")
                      for i in range(n_if)]
                pu = [ps.tile([P, TB], F32, tag="ps", name=f"z_pu{i}_{tb}")
                      for i in range(n_if)]
                for hh in range(n_hh):
                    first, last = hh == 0, hh == n_hh - 1
                    for i in range(n_if):
                        for half, pp in ((0, pg), (1, pu)):
                            lhsT = w1_lhsT(wb, hh, half, i)
                            nc.tensor.matmul(
                                pp[i][:], lhsT=lhsT, rhs=ht_rhs(hh, tb),
                                start=first, stop=last,
                            )
                for i in range(n_if):
                    swiglu(i, tb, pg[i], pu[i])
                cast_mid(tb)

        # ---- W2 stripes: DMA f32 (SP) + DVE cast into small ring ----
        b2 = {}

        def w2_stripe(hb, f):
            s = w2fp.tile([P, TB], F32, tag="w2f", name=f"w2f_{hb}_{f}")
            nc.sync.dma_start(
                s[:], w2_d[f * P : (f + 1) * P, hb * TB : (hb + 1) * TB]
            )
            b = b2p.tile([P, TB], BF16, tag="b2", name=f"b2_{hb}_{f}")
            nc.vector.tensor_copy(out=b[:], in_=s[:])
            b2[(hb, f)] = b

        # ================= emission =================
        # Startup DMA priority: h rows first (Activation queue), W1 fb0
        # on SP; fb1's DMAs are NOT issued until fb0's matmuls are
        # emitted, so they can't steal HBM bandwidth from the h rows.
        for ti in range(4):
            for c in range(n_hc):
                load_h(ti, c)
        st0 = load_w1_block(0)
        for ti in range(4, n_tt):
            for c in range(n_hc):
                load_h(ti, c)
        for ti in range(4):
            for c in range(n_hc):
                cast_h(ti, c)
        for ti in range(4):
            for c in range(n_hc):
                transpose_h(ti, c)
        wb0 = [cast_w1(0, st0, hh) for hh in range(n_hh)]
        for ti in range(4, n_tt):
            for c in range(n_hc):
                cast_h(ti, c)
        for ti in range(4, n_tt):
            for c in range(n_hc):
                transpose_h(ti, c)

        st = {1: load_w1_block(1)}
        wb = {0: wb0, 1: [None] * n_hh}

        def make_cast_mid(fb_next):
            def cast_mid(i):
                if fb_next >= n_fb:
                    return
                for hh in range(i * 8, (i + 1) * 8):
                    wb[fb_next][hh] = cast_w1(fb_next, st[fb_next], hh)
            return cast_mid

        mm1_fb0(wb0, make_cast_mid(1))
        for fb in range(1, n_fb):
            if fb + 1 < n_fb:
                st[fb + 1] = load_w1_block(fb + 1)
                wb[fb + 1] = [None] * n_hh
            if fb == n_fb - 2:
                # W2 hb0 prefetch (capped below the b2 ring depth so the
                # casts can never block the DVE queue pre-phase-C)
                for f in range(n_ft // 2):
                    w2_stripe(0, f)
            mm1_i_major(fb, wb[fb], make_cast_mid(fb + 1))

        # ---- Phase C: out = act @ W2, f-outer / tt-inner, 8 psum banks ----
        def drain(hb, tt, po):
            ob = outp.tile([P, TB], F32, tag="outp", name=f"ob{hb}_{tt}")
            nc.scalar.activation(ob[:], po[:], ACT_COPY)
            nc.sync.dma_start(
                out_d[tt * P : (tt + 1) * P, hb * TB : (hb + 1) * TB], ob[:]
            )

        def act_lhsT(f, tt):
            return act[:, f * T + tt * P : f * T + (tt + 1) * P]

        for hb in range(n_hb - 1):
            po = [ps.tile([P, TB], F32, tag="ps", name=f"po{hb}_{tt}")
                  for tt in range(n_tt)]
            for f in range(n_ft):
                if hb == 0 and f < n_ft // 2:
                    w2_stripe(0, f + n_ft // 2)
                if hb + 1 < n_hb:
                    w2_stripe(hb + 1, f)
                first, last = f == 0, f == n_ft - 1
                rhs = b2[(hb, f)][:]
                for tt in range(n_tt):
                    nc.tensor.matmul(
                        po[tt][:], lhsT=act_lhsT(f, tt), rhs=rhs,
                        start=first, stop=last,
                    )
            for tt in range(n_tt):
                drain(hb, tt, po[tt])

        # last hb: tt-outer so psum drains stagger into a short tail
        hb = n_hb - 1
        for tt in range(n_tt):
            po = ps.tile([P, TB], F32, tag="ps", name=f"po{hb}_{tt}")
            for f in range(n_ft):
                nc.tensor.matmul(
                    po[:], lhsT=act_lhsT(f, tt), rhs=b2[(hb, f)][:],
                    start=(f == 0), stop=(f == n_ft - 1),
                )
            drain(hb, tt, po)


def build_nc(T=T, H=H, F=F):
    nc = bacc.Bacc(
        "TRN2", target_bir_lowering=False, debug=False, enable_asserts=False
    )
    with tile.TileContext(nc) as tc:
        build_kernel_body(tc, T=T, H=H, F=F)
    nc.compile()
    return nc


_NC_CACHE = None


def run(hidden_states, gate_up_proj, down_proj, trace=False, **kw):
    """Run on the 8 NeuronCores; returns (output, BassKernelResults)."""
    global _NC_CACHE
    if _NC_CACHE is None:
        _NC_CACHE = build_nc()
    nc = _NC_CACHE

    hs = np.ascontiguousarray(np.asarray(hidden_states), dtype=np.float32)
    gup = np.ascontiguousarray(np.asarray(gate_up_proj), dtype=np.float32)
    dp = np.ascontiguousarray(np.asarray(down_proj), dtype=np.float32)
    assert hs.shape == (N_CORES * T, H), hs.shape
    assert gup.shape == (N_CORES, H, 2 * F), gup.shape
    assert dp.shape == (N_CORES, F, H), dp.shape

    in_maps = [
        {
            "hidden_states": np.ascontiguousarray(hs[i * T : (i + 1) * T]),
            "gate_up_proj": np.ascontiguousarray(gup[i]),
            "down_proj": np.ascontiguousarray(dp[i]),
        }
        for i in range(N_CORES)
    ]
    res = run_bass_kernel_spmd(
        nc, in_maps, core_ids=list(range(N_CORES)), trace=trace, **kw
    )
    out = np.concatenate(
        [res.results[i]["out"] for i in range(N_CORES)], axis=0
    ).astype(np.float32)
    return out, res


def kernel(hidden_states, gate_up_proj, down_proj):
    out, _ = run(hidden_states, gate_up_proj, down_proj, trace=False)
    return out
